# revision 2
# baseline (speedup 1.0000x reference)
"""Trainium2 Bass kernel v4: Gaussian-RBF basis expansion + batched matmul.

Computes, for B=32 batches, N=65536 positions, DEG=32 basis functions,
D=8 output dims:
    basis[b,n,g] = exp(-(x[b,n] - c_g)^2 / (2*0.04))
    result[b,n,d] = sum_g basis[b,n,g] * weights[b,d,g]
and returns (result, zeros_like(result)).

v4 structure (v3 measured 84us; PE busy 51us with 9us warm, DVE 38, ACT 34,
16 DMA queues 60-100% busy on descriptor overhead, 28us pipeline fill):
  * All inputs fp16, derived from xh = fp16(x) so the kernel computes f(xh)
    exactly: xc shrinks 96->64 rows (3->2 MiB), xpk fp32->fp16 (1->0.5 MiB).
    Coefficient 25c split fp16-hi + residual on a duplicated xh row; the
    residual's rounding is recentered into the activation bias.
  * e-partition layout m = b*8+gg (batch-major) so the per-tile T8
    replication 16->128 partitions is ONE DMA with a stride-0 free dim
    (validated on HW), not 8 small DMAs: 64 -> 8 dma_starts.
  * j-outer E-mm order: chunk j=0 runs without any ladder dependency; each
    ladder step is consumed just-in-time, weights loaded once per chunk.
  * PSUM as 4x [128,1024] 2-bank pairs (paA/paB args, pmA/pmB matmul);
    e0 exps and evacs operate on 1024-wide pairs (half the instructions).
  * Args lead emm by 2 tiles (args(k+2) at tile-k start), e0(k+2) early in
    tile k, ladder(k+2) spans tiles k/k+1 on DVE, evac A=ACT / B=DVE,
    2-slot osb, q-major tile order so T8-exp-half-a unblocks tiles 0-3.
"""

import numpy as np
from contextlib import ExitStack

import concourse.bass as bass
from concourse import mybir
from concourse.bass_utils import run_bass_kernel_spmd

# Problem constants (hardcoded per harness contract)
B, D, N, DEG = 32, 8, 65536, 32
SCALE = 0.04
INV2S = 1.0 / (2.0 * SCALE)  # 12.5
NCORES = 8
NSH = N // NCORES  # 8192 positions per core

# Layout constants
T2 = 2048          # positions per pipeline tile
SUB = 512          # matmul moving-free-dim (one fp32 PSUM bank)
NQ = NSH // T2     # 4 position blocks
GB = 16            # batches per group
NG = B // GB       # 2 batch groups
PG = 8             # degrees per chunk
NCHUNK = DEG // PG # 4 chunks
NIT = NG * NQ      # 8 pipeline tiles, q-major: it = q*NG + g
NSUB = T2 // SUB   # 4 sub-tiles per tile
KA = 64            # arg-matmul contraction rows (xh, xh, msqh, msql)

H = 1.01 / 31.0
T8A = 2.0 * INV2S * PG * H  # 6.51612903...: T8 = exp(T8A * x)/2

FP = mybir.dt.float32
BF = mybir.dt.bfloat16
HF = mybir.dt.float16

_centers = np.linspace(0.0, 1.01, DEG).astype(np.float64)


def _build():
    nc = bass.Bass(
        "TRN2", target_bir_lowering=False, debug=False, num_devices=NCORES
    )
    xpk_d = nc.dram_tensor("xpk", [128, T2], HF, kind="ExternalInput")
    xc_d = nc.dram_tensor("xc", [KA, NIT * T2], HF, kind="ExternalInput")
    lhsTa_d = nc.dram_tensor("lhsTa", [KA, 128], HF, kind="ExternalInput")
    lhsTw_d = nc.dram_tensor(
        "lhsTw", [128, NG, NCHUNK, 128], BF, kind="ExternalInput"
    )
    ebias_d = nc.dram_tensor("ebias", [128, 2], FP, kind="ExternalInput")
    out_d = nc.dram_tensor("out", [NG, 128, NSH], HF, kind="ExternalOutput")

    EXP = mybir.ActivationFunctionType.Exp

    with ExitStack() as ctx:
        en = ctx.enter_context
        # --- SBUF ---------------------------------------------------------
        xpk = en(nc.sbuf_tensor("xpk_sb", [128, T2], HF)).ap()
        xc = en(nc.sbuf_tensor("xc_sb", [KA, NIT * T2], HF)).ap()
        lhsTa = en(nc.sbuf_tensor("lhsTa_sb", [KA, 128], HF)).ap()
        lhsTw = en(nc.sbuf_tensor("lhsTw_sb", [128, NG, NCHUNK, 128], BF)).ap()
        ebias = en(nc.sbuf_tensor("ebias_sb", [128, 2], FP)).ap()
        dumm = en(nc.sbuf_tensor("dumm_sb", [128, 1], FP)).ap()
        t8pk = en(nc.sbuf_tensor("t8pk_sb", [128, T2], HF)).ap()
        t8r = en(nc.sbuf_tensor("t8r_sb", [128, NIT * T2], HF)).ap()
        e = [
            [en(nc.sbuf_tensor(f"e{i}_{j}", [128, T2], HF)).ap()
             for j in range(NCHUNK)]
            for i in range(3)
        ]
        osb = [en(nc.sbuf_tensor(f"osb{i}", [128, T2], HF)).ap() for i in range(2)]
        # --- PSUM: 2 arg pairs + 2 matmul pairs (2 banks each) -----------
        pa = [en(nc.psum_tensor(f"pa{s}", [128, 2 * SUB], FP)).ap() for s in range(2)]
        pm = [en(nc.psum_tensor(f"pm{s}", [128, 2 * SUB], FP)).ap() for s in range(2)]
        # --- semaphores ---------------------------------------------------
        s_leb = en(nc.semaphore("s_leb"))  # ebias landed
        s_xpk = en(nc.semaphore("s_xpk"))  # xpk landed
        s_lwa = en(nc.semaphore("s_lwa"))  # lhsTa landed
        s_xc0 = en(nc.semaphore("s_xc0"))  # xc tile 0
        s_xc1 = en(nc.semaphore("s_xc1"))  # xc tile 1
        s_xcR = en(nc.semaphore("s_xcR"))  # xc tiles 2-7
        s_lww = en(nc.semaphore("s_lww"))  # lhsTw landed
        s_t8g = en(nc.semaphore("s_t8g"))  # T8 exp halves done (+1 each)
        s_t8r = [en(nc.semaphore(f"s_t8r{i}")) for i in range(4)]  # rep DMAs
        s_arg = en(nc.semaphore("s_arg"))  # PE arg subtile (+1; 4/tile)
        s_e0 = en(nc.semaphore("s_e0"))    # ACT e0 pair (+1; 2/tile)
        s_lad = en(nc.semaphore("s_lad"))  # DVE ladder step (+1; 3/tile)
        s_mm = en(nc.semaphore("s_mm"))    # PE emm half (+1; 2/tile)
        s_eva = en(nc.semaphore("s_eva"))  # ACT evac pair A (+1/tile)
        s_evb = en(nc.semaphore("s_evb"))  # DVE evac pair B (+1/tile)
        s_out = [en(nc.semaphore(f"s_out{i}")) for i in range(2)]  # out by slot

        def gq(it):
            return it % NG, it // NG  # g, q   (q-major order)

        with nc.Block() as block:

            @block.sync
            def _(sync):
                sync.dma_start(out=lhsTa, in_=lhsTa_d.ap()).then_inc(s_lwa, 16)
                sync.dma_start(out=xc[:, 0:T2], in_=xc_d.ap()[:, 0:T2]
                               ).then_inc(s_xc0, 16)
                sync.dma_start(out=xc[:, T2:2 * T2], in_=xc_d.ap()[:, T2:2 * T2]
                               ).then_inc(s_xc1, 16)
                sync.dma_start(out=lhsTw, in_=lhsTw_d.ap()).then_inc(s_lww, 16)
                sync.dma_start(out=xc[:, 2 * T2:], in_=xc_d.ap()[:, 2 * T2:]
                               ).then_inc(s_xcR, 16)

                def rep(it):
                    g, q = gq(it)
                    r0 = 32 * q + GB * g
                    src = t8pk[r0:r0 + GB, :].unsqueeze(1).broadcast_to(
                        [GB, 8, T2]
                    )
                    sync.dma_start(
                        out=t8r[:, T2 * it: T2 * (it + 1)], in_=src
                    ).then_inc(s_t8r[it % 4], 16)

                sync.wait_ge(s_t8g, 1)   # T8 rows 0:64  -> tiles 0-3
                for it in range(4):
                    rep(it)
                sync.wait_ge(s_t8g, 2)   # T8 rows 64:128 -> tiles 4-7
                for it in range(4, NIT):
                    rep(it)

                for k in range(NIT):
                    g, q = gq(k)
                    sync.wait_ge(s_eva, k + 1)
                    sync.wait_ge(s_evb, k + 1)
                    sync.dma_start(
                        out=out_d.ap()[g, :, T2 * q: T2 * (q + 1)],
                        in_=osb[k % 2],
                    ).then_inc(s_out[k % 2], 16)

            @block.scalar
            def _(scalar):
                # dummy exp triggers the ACT table load while inputs stream
                scalar.activation(dumm, dumm, EXP, scale=0.0)
                scalar.dma_start(out=ebias, in_=ebias_d.ap()).then_inc(s_leb, 16)
                scalar.dma_start(out=xpk, in_=xpk_d.ap()).then_inc(s_xpk, 16)
                scalar.wait_ge(s_xpk, 16)
                scalar.wait_ge(s_leb, 16)
                # T8 halves: rows 0:64 cover q0/q1 (tiles 0-3) q-major
                scalar.activation(
                    t8pk[0:64, :], xpk[0:64, :], EXP, scale=T8A,
                    bias=ebias[0:64, 1:2],
                ).then_inc(s_t8g, 1)

                def e0p(it, h):
                    # e0 pair h (0: subtiles 0-1 from paA, 1: 2-3 from paB)
                    bi = it % 3
                    scalar.wait_ge(s_arg, 4 * it + 2 * (h + 1))
                    if it >= 3 and h == 0:
                        scalar.wait_ge(s_mm, 2 * it - 4)  # e-buf WAR
                    scalar.activation(
                        e[bi][0][:, 1024 * h: 1024 * (h + 1)], pa[h],
                        EXP, scale=1.0, bias=ebias[:, 0:1],
                    ).then_inc(s_e0, 1)

                def ev_a(k):
                    scalar.wait_ge(s_mm, 2 * k + 1)
                    if k >= 2:
                        scalar.wait_ge(s_out[k % 2], 16 * (k // 2))
                    scalar.copy(osb[k % 2][:, 0:1024], pm[0]).then_inc(s_eva, 1)

                e0p(0, 0)
                e0p(0, 1)
                e0p(1, 0)
                e0p(1, 1)
                scalar.activation(
                    t8pk[64:128, :], xpk[64:128, :], EXP, scale=T8A,
                    bias=ebias[64:128, 1:2],
                ).then_inc(s_t8g, 1)
                for k in range(NIT):
                    ev_a(k)
                    if k + 2 < NIT:
                        e0p(k + 2, 0)
                        e0p(k + 2, 1)

            @block.vector
            def _(vector):
                def ladder(it):
                    bi = it % 3
                    t8v = t8r[:, T2 * it: T2 * (it + 1)]
                    vector.wait_ge(s_e0, 2 * (it + 1))
                    vector.wait_ge(s_t8r[it % 4], 16 * (it // 4 + 1))
                    if it >= 3:
                        vector.wait_ge(s_mm, 2 * it - 4)  # e-buf WAR
                    vector.tensor_mul(e[bi][1], e[bi][0], t8v).then_inc(s_lad, 1)
                    vector.tensor_mul(e[bi][2], e[bi][1], t8v).then_inc(s_lad, 1)
                    vector.tensor_mul(e[bi][3], e[bi][2], t8v).then_inc(s_lad, 1)

                def ev_b(k):
                    vector.wait_ge(s_mm, 2 * k + 2)
                    if k >= 2:
                        vector.wait_ge(s_out[k % 2], 16 * (k // 2))
                    vector.tensor_copy(
                        osb[k % 2][:, 1024:2048], pm[1]
                    ).then_inc(s_evb, 1)

                ladder(0)
                ladder(1)
                for k in range(NIT):
                    ev_b(k)
                    if k + 2 < NIT:
                        ladder(k + 2)

            @block.tensor
            def _(tensor):
                # warm-up: ramp the PE pstate while inputs stream (reads
                # uninitialized SBUF, writes pmA; first real matmul has
                # start=True so the garbage never survives)
                def warm(n):
                    for _ in range(n):
                        tensor.matmul(
                            pm[0][:, 0:256], osb[1][:, 0:128], osb[0][:, 0:256],
                            start=True, stop=True, skip_group_check=True,
                        )

                def args(it):
                    g, q = gq(it)
                    for s in range(NSUB):
                        if it == 0 and s == 0:
                            tensor.wait_ge(s_lwa, 16)
                            tensor.wait_ge(s_xc0, 16)
                        if it == 1 and s == 0:
                            tensor.wait_ge(s_xc1, 16)
                        if it == 2 and s == 0:
                            tensor.wait_ge(s_xcR, 16)
                        if it >= 1 and s % 2 == 0:
                            # pa pair WAR: e0(it-1) pair consumed
                            tensor.wait_ge(s_e0, 2 * it - 1 + s // 2)
                        c0 = T2 * it + SUB * s
                        tensor.matmul(
                            pa[s // 2][:, SUB * (s % 2): SUB * (s % 2 + 1)],
                            lhsTa, xc[:, c0: c0 + SUB],
                            start=True, stop=True, skip_group_check=True,
                        ).then_inc(s_arg, 1)

                def emm(k):
                    bi = k % 3
                    g, _ = gq(k)
                    for j in range(NCHUNK):
                        for s in range(NSUB):
                            if j == 0:
                                if s == 0:
                                    if k == 0:
                                        tensor.wait_ge(s_lww, 16)
                                    tensor.wait_ge(s_e0, 2 * k + 1)
                                    if k >= 1:
                                        tensor.wait_ge(s_eva, k)  # pmA WAR
                                if s == 2:
                                    tensor.wait_ge(s_e0, 2 * k + 2)
                                    if k >= 1:
                                        tensor.wait_ge(s_evb, k)  # pmB WAR
                            elif s == 0:
                                tensor.wait_ge(s_lad, 3 * k + j)
                            mm = tensor.matmul(
                                pm[s // 2][:, SUB * (s % 2): SUB * (s % 2 + 1)],
                                lhsTw[:, g, j, :],
                                e[bi][j][:, SUB * s: SUB * (s + 1)],
                                start=(j == 0), stop=(j == NCHUNK - 1),
                                skip_group_check=True,
                            )
                            if j == NCHUNK - 1 and s == 1:
                                mm.then_inc(s_mm, 1)  # pmA complete
                            if j == NCHUNK - 1 and s == 3:
                                mm.then_inc(s_mm, 1)  # pmB complete
                    # emm order is j-outer but subtile-paired per j so pmA
                    # (s0,s1) completes at j=3,s=1 and pmB at j=3,s=3

                warm(12)
                args(0)
                warm(4)
                args(1)
                for k in range(NIT):
                    emm(k)
                    if k + 2 < NIT:
                        args(k + 2)
    return nc


def _host_inputs(weights, positions):
    """Per-core in_maps: fp16 packing; everything derived from fp16(x)."""
    import ml_dtypes

    bf = ml_dtypes.bfloat16
    hf = np.float16
    w = np.ascontiguousarray(np.asarray(weights, dtype=np.float32))
    x = np.ascontiguousarray(np.asarray(positions, dtype=np.float32))
    cent = _centers

    # coefficient 25c split fp16-hi + residual, recentered into the bias
    coef = 2.0 * INV2S * cent[:PG]                   # f64
    chi = np.asarray(coef, np.float32).astype(hf).astype(np.float64)
    clo = np.asarray(coef - chi, np.float32).astype(hf).astype(np.float64)
    delta = (chi + clo) - coef

    ggm = np.arange(128) % PG    # gg of e-partition m = b*8+gg
    bm = np.arange(128) // PG    # b  of e-partition m

    lhsTa = np.zeros((KA, 128), np.float32)
    for k in range(GB):
        sel = bm == k
        lhsTa[k, sel] = chi[ggm[sel]].astype(np.float32)
        lhsTa[GB + k, sel] = clo[ggm[sel]].astype(np.float32)
        lhsTa[2 * GB + k, sel] = 1.0   # msqh
        lhsTa[3 * GB + k, sel] = 1.0   # msql
    lhsTa = np.ascontiguousarray(lhsTa.astype(hf))

    ebias = np.zeros((128, 2), np.float32)
    bias0 = -INV2S * cent[:PG] ** 2 - delta * cent[:PG]
    ebias[:, 0] = bias0[ggm].astype(np.float32)
    ebias[:, 1] = np.float32(np.log(0.5))
    ebias = np.ascontiguousarray(ebias)

    # E-matmul weights: ladder rescale + 2^j compensation folded in
    jj = np.arange(NCHUNK)[:, None]
    gg = np.arange(PG)[None, :]
    fac = np.exp(-INV2S * (cent[PG * jj + gg] ** 2 - cent[gg] ** 2))
    fac = fac * (2.0 ** np.arange(NCHUNK))[:, None]
    w4 = w.reshape(NG, GB, D, NCHUNK, PG).astype(np.float64)
    w4 = w4 * fac[None, None, None, :, :]            # [g, b, d, j, gg]
    lhsTw = np.zeros((128, NG, NCHUNK, 128), np.float64)
    for b in range(GB):
        # K row = b*8+gg ; out col = d*16+b
        lhsTw[b * PG:(b + 1) * PG, :, :, b::GB] = w4[:, b].transpose(3, 0, 2, 1)
    lhsTw = np.ascontiguousarray(lhsTw.astype(np.float32).astype(bf))

    in_maps = []
    for ci in range(NCORES):
        xs = x[:, ci * NSH: (ci + 1) * NSH]          # [32, NSH]
        xh = xs.astype(hf)                           # THE x: everything uses xh
        xpk = np.ascontiguousarray(
            xh.reshape(B, NQ, T2).transpose(1, 0, 2).reshape(128, T2)
        )
        xh64 = xh.astype(np.float64)
        msq = -INV2S * xh64 * xh64
        msqh = msq.astype(np.float32).astype(hf)
        msql = (msq - msqh.astype(np.float64)).astype(np.float32).astype(hf)
        xct = np.zeros((KA, NIT, T2), hf)
        for it in range(NIT):
            g, q = it % NG, it // NG
            rows = slice(GB * g, GB * (g + 1))
            cols = slice(T2 * q, T2 * (q + 1))
            xct[0:GB, it] = xh[rows, cols]
            xct[GB:2 * GB, it] = xh[rows, cols]
            xct[2 * GB:3 * GB, it] = msqh[rows, cols]
            xct[3 * GB:4 * GB, it] = msql[rows, cols]
        in_maps.append(
            {
                "xpk": xpk,
                "xc": np.ascontiguousarray(xct.reshape(KA, NIT * T2)),
                "lhsTa": lhsTa,
                "lhsTw": lhsTw,
                "ebias": ebias,
            }
        )
    return in_maps


def _gather(results):
    """[NG, 128, NSH] per core, rows m=d*16+b  ->  full [B, N, D]."""
    outs = []
    for r in results:
        o = r["out"].astype(np.float32).reshape(NG, D, GB, NSH)  # [g, d, b, n]
        outs.append(o.transpose(0, 2, 3, 1).reshape(B, NSH, D))  # [b, n, d]
    full = np.concatenate(outs, axis=1)  # [B, N, D]
    return np.ascontiguousarray(full)


_NC_CACHE = {}


def run(inputs, trace=False, **trace_kwargs):
    """Builds (cached), runs on 8 cores, returns ((result, zeros), results)."""
    key = ("v4",)
    if key not in _NC_CACHE:
        _NC_CACHE[key] = _build()
    nc = _NC_CACHE[key]
    in_maps = _host_inputs(inputs["weights"], inputs["positions"])
    br = run_bass_kernel_spmd(
        nc, in_maps, list(range(NCORES)), trace=trace, **trace_kwargs
    )
    result = _gather(br.results)
    return (result, np.zeros_like(result)), br


def kernel(weights, weights_std, positions):
    out, _ = run(
        {"weights": weights, "weights_std": weights_std, "positions": positions}
    )
    return out


# revision 3
# speedup vs baseline: 1.3276x; 1.3276x over previous
"""Trainium2 Bass kernel v5: Gaussian-RBF basis expansion + batched matmul.

Computes, for B=32 batches, N=65536 positions, DEG=32 basis functions,
D=8 output dims:
    basis[b,n,g] = exp(-(x[b,n] - c_g)^2 / (2*0.04))
    result[b,n,d] = sum_g basis[b,n,g] * weights[b,d,g]
and returns (result, zeros_like(result)).

v5 structure (v4 measured 94us: ladder chain started ~5us late per tile ->
~3.3us/tile of PE stalls; v3 84us):
  * All inputs fp16 derived from xh = fp16(x): xc 64 rows (2 MiB), xpk fp16
    (0.5 MiB). Coefficient 25c split fp16-hi + residual on a duplicated xh
    row, residual rounding recentered into the activation bias.
  * e-partition layout m = b*8+gg so per-tile T8 replication 16->128
    partitions is ONE stride-0-broadcast DMA (8 dma_starts total, not 64).
  * Pipeline per tile k: PE = [args(k+2) x4, emm(k) s-outer (4 same-bank
    accumulating matmuls per subtile)]; ACT = [e0(k+2) x4 512-wide,
    evac(k) s0,s1,s2]; DVE = [ladder(k+2) as two half-tile 3-step chains,
    evac(k) s3]. Ladder leads its consumer by ~2 tiles -> no PE stalls.
  * Tile 0 emm is j-outer so chunk-0 matmuls run before the first ladder
    lands. q-major tile order + split T8 exp unblocks replication early.
  * PSUM: pa/pm as 2x [128,1024] 2-bank pairs (matmuls write 512 slices).
"""

import numpy as np
from contextlib import ExitStack

import concourse.bass as bass
from concourse import mybir
from concourse.bass_utils import run_bass_kernel_spmd

# Problem constants (hardcoded per harness contract)
B, D, N, DEG = 32, 8, 65536, 32
SCALE = 0.04
INV2S = 1.0 / (2.0 * SCALE)  # 12.5
NCORES = 8
NSH = N // NCORES  # 8192 positions per core

# Layout constants
T2 = 2048          # positions per pipeline tile
SUB = 512          # matmul moving-free-dim (one fp32 PSUM bank)
NQ = NSH // T2     # 4 position blocks
GB = 16            # batches per group
NG = B // GB       # 2 batch groups
PG = 8             # degrees per chunk
NCHUNK = DEG // PG # 4 chunks
NIT = NG * NQ      # 8 pipeline tiles, q-major: it = q*NG + g
NSUB = T2 // SUB   # 4 sub-tiles per tile
KA = 64            # arg-matmul contraction rows (xh, xh, msqh, msql)

H = 1.01 / 31.0
T8A = 2.0 * INV2S * PG * H  # 6.51612903...: T8 = exp(T8A * x)/2

FP = mybir.dt.float32
BF = mybir.dt.bfloat16
HF = mybir.dt.float16

_centers = np.linspace(0.0, 1.01, DEG).astype(np.float64)


def _build():
    nc = bass.Bass(
        "TRN2", target_bir_lowering=False, debug=False, num_devices=NCORES
    )
    xpk_d = nc.dram_tensor("xpk", [128, T2], HF, kind="ExternalInput")
    xc_d = nc.dram_tensor("xc", [KA, NIT * T2], HF, kind="ExternalInput")
    lhsTa_d = nc.dram_tensor("lhsTa", [KA, 128], HF, kind="ExternalInput")
    lhsTw_d = nc.dram_tensor(
        "lhsTw", [128, NG, NCHUNK, 128], BF, kind="ExternalInput"
    )
    ebias_d = nc.dram_tensor("ebias", [128, 2], FP, kind="ExternalInput")
    out_d = nc.dram_tensor("out", [NG, 128, NSH], HF, kind="ExternalOutput")

    EXP = mybir.ActivationFunctionType.Exp

    with ExitStack() as ctx:
        en = ctx.enter_context
        # --- SBUF ---------------------------------------------------------
        xpk = en(nc.sbuf_tensor("xpk_sb", [128, T2], HF)).ap()
        xc = en(nc.sbuf_tensor("xc_sb", [KA, NIT * T2], HF)).ap()
        lhsTa = en(nc.sbuf_tensor("lhsTa_sb", [KA, 128], HF)).ap()
        lhsTw = en(nc.sbuf_tensor("lhsTw_sb", [128, NG, NCHUNK, 128], BF)).ap()
        ebias = en(nc.sbuf_tensor("ebias_sb", [128, 2], FP)).ap()
        dumm = en(nc.sbuf_tensor("dumm_sb", [128, 1], FP)).ap()
        t8pk = en(nc.sbuf_tensor("t8pk_sb", [128, T2], HF)).ap()
        t8r = en(nc.sbuf_tensor("t8r_sb", [128, NIT * T2], HF)).ap()
        e = [
            [en(nc.sbuf_tensor(f"e{i}_{j}", [128, T2], HF)).ap()
             for j in range(NCHUNK)]
            for i in range(3)
        ]
        osb = [en(nc.sbuf_tensor(f"osb{i}", [128, T2], HF)).ap() for i in range(2)]
        # --- PSUM: 2 arg pairs + 2 matmul pairs (2 banks each) -----------
        pa = [en(nc.psum_tensor(f"pa{s}", [128, 2 * SUB], FP)).ap() for s in range(2)]
        pm = [en(nc.psum_tensor(f"pm{s}", [128, 2 * SUB], FP)).ap() for s in range(2)]

        def pav(s):
            return pa[s // 2][:, SUB * (s % 2): SUB * (s % 2 + 1)]

        def pmv(s):
            return pm[s // 2][:, SUB * (s % 2): SUB * (s % 2 + 1)]

        # --- semaphores ---------------------------------------------------
        s_leb = en(nc.semaphore("s_leb"))
        s_xpk = en(nc.semaphore("s_xpk"))
        s_lwa = en(nc.semaphore("s_lwa"))
        s_xc0 = en(nc.semaphore("s_xc0"))
        s_xc1 = en(nc.semaphore("s_xc1"))
        s_xcR = en(nc.semaphore("s_xcR"))
        s_lww = en(nc.semaphore("s_lww"))
        s_t8g = en(nc.semaphore("s_t8g"))  # T8 exp halves (+1 each)
        s_t8r = [en(nc.semaphore(f"s_t8r{i}")) for i in range(4)]
        s_arg = en(nc.semaphore("s_arg"))  # +1 per arg subtile (4/tile)
        s_e0 = en(nc.semaphore("s_e0"))    # +1 per e0 subtile (4/tile)
        s_lad = en(nc.semaphore("s_lad"))  # +1 per half-ladder step (6/tile)
        s_mm = en(nc.semaphore("s_mm"))    # +1 per emm s-group (4/tile)
        s_eva = en(nc.semaphore("s_eva"))  # +1 per ACT evac (3/tile: s0,s1,s2)
        s_evb = en(nc.semaphore("s_evb"))  # +1 per DVE evac (1/tile: s3)
        s_out = [en(nc.semaphore(f"s_out{i}")) for i in range(2)]

        def gq(it):
            return it % NG, it // NG  # g, q   (q-major order)

        with nc.Block() as block:

            @block.sync
            def _(sync):
                sync.dma_start(out=lhsTa, in_=lhsTa_d.ap()).then_inc(s_lwa, 16)
                sync.dma_start(out=xc[:, 0:T2], in_=xc_d.ap()[:, 0:T2]
                               ).then_inc(s_xc0, 16)
                sync.dma_start(out=xc[:, T2:2 * T2], in_=xc_d.ap()[:, T2:2 * T2]
                               ).then_inc(s_xc1, 16)
                sync.dma_start(out=lhsTw, in_=lhsTw_d.ap()).then_inc(s_lww, 16)
                sync.dma_start(out=xc[:, 2 * T2:], in_=xc_d.ap()[:, 2 * T2:]
                               ).then_inc(s_xcR, 16)

                def rep(it):
                    g, q = gq(it)
                    r0 = 32 * q + GB * g
                    src = t8pk[r0:r0 + GB, :].unsqueeze(1).broadcast_to(
                        [GB, 8, T2]
                    )
                    sync.dma_start(
                        out=t8r[:, T2 * it: T2 * (it + 1)], in_=src
                    ).then_inc(s_t8r[it % 4], 16)

                sync.wait_ge(s_t8g, 1)   # T8 rows 0:64  -> tiles 0-3
                for it in range(4):
                    rep(it)
                sync.wait_ge(s_t8g, 2)   # T8 rows 64:128 -> tiles 4-7
                for it in range(4, NIT):
                    rep(it)

                for k in range(NIT):
                    g, q = gq(k)
                    sync.wait_ge(s_eva, 3 * k + 3)
                    sync.wait_ge(s_evb, k + 1)
                    sync.dma_start(
                        out=out_d.ap()[g, :, T2 * q: T2 * (q + 1)],
                        in_=osb[k % 2],
                    ).then_inc(s_out[k % 2], 16)

            @block.scalar
            def _(scalar):
                # dummy exp triggers the ACT table load while inputs stream
                scalar.activation(dumm, dumm, EXP, scale=0.0)
                scalar.dma_start(out=ebias, in_=ebias_d.ap()).then_inc(s_leb, 16)
                scalar.dma_start(out=xpk, in_=xpk_d.ap()).then_inc(s_xpk, 16)
                scalar.wait_ge(s_xpk, 16)
                scalar.wait_ge(s_leb, 16)
                # T8 halves: rows 0:64 cover q0/q1 (tiles 0-3) q-major
                scalar.activation(
                    t8pk[0:64, :], xpk[0:64, :], EXP, scale=T8A,
                    bias=ebias[0:64, 1:2],
                ).then_inc(s_t8g, 1)

                def e0s(it, s):
                    bi = it % 3
                    scalar.wait_ge(s_arg, 4 * it + s + 1)
                    if it >= 3 and s == 0:
                        scalar.wait_ge(s_mm, 4 * it - 8)  # e-buf WAR
                    scalar.activation(
                        e[bi][0][:, SUB * s: SUB * (s + 1)], pav(s),
                        EXP, scale=1.0, bias=ebias[:, 0:1],
                    ).then_inc(s_e0, 1)

                def ev_a(k, s):
                    scalar.wait_ge(s_mm, 4 * k + s + 1)
                    if k >= 2 and s == 0:
                        scalar.wait_ge(s_out[k % 2], 16 * (k // 2))
                    scalar.copy(
                        osb[k % 2][:, SUB * s: SUB * (s + 1)], pmv(s)
                    ).then_inc(s_eva, 1)

                for s in range(NSUB):
                    e0s(0, s)
                for s in range(NSUB):
                    e0s(1, s)
                scalar.activation(
                    t8pk[64:128, :], xpk[64:128, :], EXP, scale=T8A,
                    bias=ebias[64:128, 1:2],
                ).then_inc(s_t8g, 1)
                for k in range(NIT):
                    if k + 2 < NIT:
                        for s in range(NSUB):
                            e0s(k + 2, s)
                    for s in (0, 1, 2):
                        ev_a(k, s)

            @block.vector
            def _(vector):
                def ladder(it, h):
                    # half-tile ladder: cols [1024h : 1024(h+1)]
                    bi = it % 3
                    cs = slice(1024 * h, 1024 * (h + 1))
                    t8v = t8r[:, T2 * it + 1024 * h: T2 * it + 1024 * (h + 1)]
                    vector.wait_ge(s_e0, 4 * it + 2 * (h + 1))
                    if h == 0:
                        vector.wait_ge(s_t8r[it % 4], 16 * (it // 4 + 1))
                        if it >= 3:
                            vector.wait_ge(s_mm, 4 * it - 8)  # e-buf WAR
                    vector.tensor_mul(e[bi][1][:, cs], e[bi][0][:, cs], t8v
                                      ).then_inc(s_lad, 1)
                    vector.tensor_mul(e[bi][2][:, cs], e[bi][1][:, cs], t8v
                                      ).then_inc(s_lad, 1)
                    vector.tensor_mul(e[bi][3][:, cs], e[bi][2][:, cs], t8v
                                      ).then_inc(s_lad, 1)

                def ev_b(k):
                    vector.wait_ge(s_mm, 4 * k + 4)
                    if k >= 2:
                        vector.wait_ge(s_out[k % 2], 16 * (k // 2))
                    vector.tensor_copy(
                        osb[k % 2][:, 3 * SUB: 4 * SUB], pmv(3)
                    ).then_inc(s_evb, 1)

                ladder(0, 0)
                ladder(0, 1)
                ladder(1, 0)
                ladder(1, 1)
                for k in range(NIT):
                    if k + 2 < NIT:
                        ladder(k + 2, 0)
                        ladder(k + 2, 1)
                    ev_b(k)

            @block.tensor
            def _(tensor):
                # warm-up: ramp the PE pstate while inputs stream (reads
                # uninitialized SBUF, writes pmA; first real matmul has
                # start=True so the garbage never survives)
                def warm(n):
                    for _ in range(n):
                        tensor.matmul(
                            pm[0][:, 0:256], osb[1][:, 0:128], osb[0][:, 0:256],
                            start=True, stop=True, skip_group_check=True,
                        )

                def args(it):
                    for s in range(NSUB):
                        if it == 0 and s == 0:
                            tensor.wait_ge(s_lwa, 16)
                            tensor.wait_ge(s_xc0, 16)
                        if it == 1 and s == 0:
                            tensor.wait_ge(s_xc1, 16)
                        if it == 2 and s == 0:
                            tensor.wait_ge(s_xcR, 16)
                        if it >= 1 and s % 2 == 0:
                            # pa pair WAR: e0(it-1) pair consumed
                            tensor.wait_ge(s_e0, 4 * it - 2 + s // 2 * 2)
                        c0 = T2 * it + SUB * s
                        tensor.matmul(
                            pav(s), lhsTa, xc[:, c0: c0 + SUB],
                            start=True, stop=True, skip_group_check=True,
                        ).then_inc(s_arg, 1)

                def emm_s_outer(k):
                    bi = k % 3
                    g, _ = gq(k)
                    for s in range(NSUB):
                        for j in range(NCHUNK):
                            if j == 0:
                                tensor.wait_ge(s_e0, 4 * k + s + 1)
                                if s == 0 and k >= 1:
                                    tensor.wait_ge(s_eva, 3 * k - 1)
                                if s == 2 and k >= 1:
                                    tensor.wait_ge(s_eva, 3 * k)
                                if s == 3 and k >= 1:
                                    tensor.wait_ge(s_evb, k)
                            elif s in (0, 2):
                                # half h=s//2 ladder step j
                                tensor.wait_ge(s_lad, 6 * k + 3 * (s // 2) + j)
                            mm = tensor.matmul(
                                pmv(s), lhsTw[:, g, j, :],
                                e[bi][j][:, SUB * s: SUB * (s + 1)],
                                start=(j == 0), stop=(j == NCHUNK - 1),
                                skip_group_check=True,
                            )
                            if j == NCHUNK - 1:
                                mm.then_inc(s_mm, 1)

                def emm_j_outer(k):
                    # tile 0 only: j=0 pass runs before any ladder output
                    bi = k % 3
                    g, _ = gq(k)
                    for j in range(NCHUNK):
                        for s in range(NSUB):
                            if j == 0:
                                tensor.wait_ge(s_e0, 4 * k + s + 1)
                                if s == 0:
                                    tensor.wait_ge(s_lww, 16)
                            elif s in (0, 2):
                                tensor.wait_ge(s_lad, 6 * k + 3 * (s // 2) + j)
                            mm = tensor.matmul(
                                pmv(s), lhsTw[:, g, j, :],
                                e[bi][j][:, SUB * s: SUB * (s + 1)],
                                start=(j == 0), stop=(j == NCHUNK - 1),
                                skip_group_check=True,
                            )
                            if j == NCHUNK - 1:
                                mm.then_inc(s_mm, 1)

                warm(12)
                args(0)
                warm(4)
                args(1)
                for k in range(NIT):
                    if k + 2 < NIT:
                        args(k + 2)
                    if k == 0:
                        emm_j_outer(k)
                    else:
                        emm_s_outer(k)
    return nc


def _host_inputs(weights, positions):
    """Per-core in_maps: fp16 packing; everything derived from fp16(x)."""
    import ml_dtypes

    bf = ml_dtypes.bfloat16
    hf = np.float16
    w = np.ascontiguousarray(np.asarray(weights, dtype=np.float32))
    x = np.ascontiguousarray(np.asarray(positions, dtype=np.float32))
    cent = _centers

    # coefficient 25c split fp16-hi + residual, recentered into the bias
    coef = 2.0 * INV2S * cent[:PG]                   # f64
    chi = np.asarray(coef, np.float32).astype(hf).astype(np.float64)
    clo = np.asarray(coef - chi, np.float32).astype(hf).astype(np.float64)
    delta = (chi + clo) - coef

    ggm = np.arange(128) % PG    # gg of e-partition m = b*8+gg
    bm = np.arange(128) // PG    # b  of e-partition m

    lhsTa = np.zeros((KA, 128), np.float32)
    for k in range(GB):
        sel = bm == k
        lhsTa[k, sel] = chi[ggm[sel]].astype(np.float32)
        lhsTa[GB + k, sel] = clo[ggm[sel]].astype(np.float32)
        lhsTa[2 * GB + k, sel] = 1.0   # msqh
        lhsTa[3 * GB + k, sel] = 1.0   # msql
    lhsTa = np.ascontiguousarray(lhsTa.astype(hf))

    ebias = np.zeros((128, 2), np.float32)
    bias0 = -INV2S * cent[:PG] ** 2 - delta * cent[:PG]
    ebias[:, 0] = bias0[ggm].astype(np.float32)
    ebias[:, 1] = np.float32(np.log(0.5))
    ebias = np.ascontiguousarray(ebias)

    # E-matmul weights: ladder rescale + 2^j compensation folded in
    jj = np.arange(NCHUNK)[:, None]
    gg = np.arange(PG)[None, :]
    fac = np.exp(-INV2S * (cent[PG * jj + gg] ** 2 - cent[gg] ** 2))
    fac = fac * (2.0 ** np.arange(NCHUNK))[:, None]
    w4 = w.reshape(NG, GB, D, NCHUNK, PG).astype(np.float64)
    w4 = w4 * fac[None, None, None, :, :]            # [g, b, d, j, gg]
    lhsTw = np.zeros((128, NG, NCHUNK, 128), np.float64)
    for b in range(GB):
        # K row = b*8+gg ; out col = d*16+b
        lhsTw[b * PG:(b + 1) * PG, :, :, b::GB] = w4[:, b].transpose(3, 0, 2, 1)
    lhsTw = np.ascontiguousarray(lhsTw.astype(np.float32).astype(bf))

    in_maps = []
    for ci in range(NCORES):
        xs = x[:, ci * NSH: (ci + 1) * NSH]          # [32, NSH]
        xh = xs.astype(hf)                           # THE x: everything uses xh
        xpk = np.ascontiguousarray(
            xh.reshape(B, NQ, T2).transpose(1, 0, 2).reshape(128, T2)
        )
        xh64 = xh.astype(np.float64)
        msq = -INV2S * xh64 * xh64
        msqh = msq.astype(np.float32).astype(hf)
        msql = (msq - msqh.astype(np.float64)).astype(np.float32).astype(hf)
        xct = np.zeros((KA, NIT, T2), hf)
        for it in range(NIT):
            g, q = it % NG, it // NG
            rows = slice(GB * g, GB * (g + 1))
            cols = slice(T2 * q, T2 * (q + 1))
            xct[0:GB, it] = xh[rows, cols]
            xct[GB:2 * GB, it] = xh[rows, cols]
            xct[2 * GB:3 * GB, it] = msqh[rows, cols]
            xct[3 * GB:4 * GB, it] = msql[rows, cols]
        in_maps.append(
            {
                "xpk": xpk,
                "xc": np.ascontiguousarray(xct.reshape(KA, NIT * T2)),
                "lhsTa": lhsTa,
                "lhsTw": lhsTw,
                "ebias": ebias,
            }
        )
    return in_maps


def _gather(results):
    """[NG, 128, NSH] per core, rows m=d*16+b  ->  full [B, N, D]."""
    outs = []
    for r in results:
        o = r["out"].astype(np.float32).reshape(NG, D, GB, NSH)  # [g, d, b, n]
        outs.append(o.transpose(0, 2, 3, 1).reshape(B, NSH, D))  # [b, n, d]
    full = np.concatenate(outs, axis=1)  # [B, N, D]
    return np.ascontiguousarray(full)


_NC_CACHE = {}


def run(inputs, trace=False, **trace_kwargs):
    """Builds (cached), runs on 8 cores, returns ((result, zeros), results)."""
    key = ("v5",)
    if key not in _NC_CACHE:
        _NC_CACHE[key] = _build()
    nc = _NC_CACHE[key]
    in_maps = _host_inputs(inputs["weights"], inputs["positions"])
    br = run_bass_kernel_spmd(
        nc, in_maps, list(range(NCORES)), trace=trace, **trace_kwargs
    )
    result = _gather(br.results)
    return (result, np.zeros_like(result)), br


def kernel(weights, weights_std, positions):
    out, _ = run(
        {"weights": weights, "weights_std": weights_std, "positions": positions}
    )
    return out
